# revision 1
# baseline (speedup 1.0000x reference)
"""Causal group-query attention on 8 Trainium2 NeuronCores.

Sharding: core c -> (batch b = c // 4, kv-group g = c % 4).
Each core owns batch element b, q-heads [4g, 4g+4) and kv-group g (n_rep = 4,
so those 4 q-heads attend to exactly kv-group g's k/v).  Every core computes
its partial o_proj output (contracting head-concat columns [512g, 512g+512)),
and the host sums the 4 partials per batch element (the "all-reduce after
o_proj" done host-side since we return full outputs anyway).

Per-core kernel (T=2048, D=2048, HS=128):
  phase A (per 512-wide t-block): stream x^T, compute Q^T/K^T/V^T projections
    on the PE (contract over D in 16 k-tiles), add bias on ACT, apply RoPE as
    (cos*q + sin*(R@q)) where R is the rotate-half permutation done as a
    128x128 matmul; V^T is transposed back to [t, hs] tiles via PE transpose.
  phase B (same t-block as q-block jq), heads processed in pairs: for each
    causally-valid 128-wide k-tile i and head h: S^T[tk, tq] = k-tile
    stationary x q^T moving (N<=512), P^T = exp(S^T/sqrt(HS)) on ACT (no max
    subtraction needed: scores are O(3)), triangular mask multiply on the
    diagonal subtile, then O^T[hs, tq] += V-stationary @ P^T and
    denom[tq] += ones^T @ P^T, both accumulated in PSUM.
  normalize: denom -> reciprocal, broadcast across partitions with a K=1
    ones matmul, multiply into O^T while evacuating PSUM.
  phase C: o_proj out[tq, d] = sum_h (O^T_h columns)-stationary @ Wo^T_h
    moving (N=512), evacuate and DMA out.

All matmuls use float32r (full-rate fp32 mode, 1 cycle/row at N>=256); every
producer of an fp32r matmul operand emits float32r-typed output (walrus
requires "rounded to FP32r" inputs). fp32r matmul outputs must sit at PSUM
partition 0 (col-group 0) -- hence the head-pair structure for denominators.
"""

import math

import numpy as np

B, T, D = 2, 2048, 2048
N_HEAD, N_GROUP = 16, 4
HS = D // N_HEAD  # 128
N_REP = N_HEAD // N_GROUP  # 4
NH_C = N_HEAD // N_GROUP  # heads per core = 4
INV_SQRT_HS = 1.0 / math.sqrt(HS)

_NC_CACHE: dict = {}


def build_nc(t=T, mm_r=True, gp_mask=False, recip_psum=True, gp_outdma=True, recip_approx=True):
    """Build and compile the per-core Bass program. Returns the compiled nc."""
    import concourse.bass as bass  # noqa: F401
    import concourse.mybir as mybir
    import concourse.tile as tile
    from concourse import bacc

    f32 = mybir.dt.float32
    f32r = mybir.dt.float32r
    ident_f = mybir.ActivationFunctionType.Identity
    exp_f = mybir.ActivationFunctionType.Exp

    def r(ap):
        # float32r view: fp32 matmul at full issue rate (1 cycle/row, N>=256)
        return ap.bitcast(f32r) if mm_r else ap

    nd = D // 128  # d-tiles (contraction) = 16
    tb_n = t // 512  # 512-wide t blocks
    nk = t // 128  # 128-wide k tiles

    nc = bacc.Bacc("TRN2", target_bir_lowering=False, debug=False)

    xd = nc.dram_tensor("x_t", [D, t], f32, kind="ExternalInput")
    wqd = nc.dram_tensor("wq_t", [D, NH_C * HS], f32, kind="ExternalInput")
    wkd = nc.dram_tensor("wk_t", [D, HS], f32, kind="ExternalInput")
    wvd = nc.dram_tensor("wv_t", [D, HS], f32, kind="ExternalInput")
    wod = nc.dram_tensor("wo_t", [NH_C * HS, D], f32, kind="ExternalInput")
    cosd = nc.dram_tensor("cos_t", [HS, t], f32, kind="ExternalInput")
    sind = nc.dram_tensor("sin_t", [HS, t], f32, kind="ExternalInput")
    bqd = nc.dram_tensor("b_q", [HS, NH_C], f32, kind="ExternalInput")
    bkd = nc.dram_tensor("b_k", [HS, 1], f32, kind="ExternalInput")
    bvd = nc.dram_tensor("b_v", [HS, 1], f32, kind="ExternalInput")
    rtd = nc.dram_tensor("r_t", [HS, HS], f32, kind="ExternalInput")
    maskd = nc.dram_tensor("mask_ut", [128, 128], f32, kind="ExternalInput")
    identd = nc.dram_tensor("ident", [128, 128], f32, kind="ExternalInput")
    outd = nc.dram_tensor("out", [t, D], f32, kind="ExternalOutput")

    with tile.TileContext(nc) as tc:
        with (
            tc.tile_pool(name="consts", bufs=1) as consts,
            tc.tile_pool(name="wpool", bufs=1) as wpool,
            tc.tile_pool(name="resid", bufs=1) as resid,
            tc.tile_pool(name="xin", bufs=6) as xin,
            tc.tile_pool(name="work", bufs=3) as work,
            tc.tile_pool(name="qfp", bufs=2) as qfp,
            tc.tile_pool(name="otp", bufs=6) as otp,
            tc.tile_pool(name="outp", bufs=4) as outp,
            tc.tile_pool(name="dscr", bufs=4, space="DRAM") as dscr,
            tc.tile_pool(name="psum", bufs=7, space="PSUM") as psum,
        ):
            def bank(name):
                return psum.tile([128, 512], f32, tag="bank", name=name)

            # ---- constants / weights (loaded once) ----
            cos_sb = consts.tile([128, t], f32, name="cos_sb")
            sin_sb = consts.tile([128, t], f32, name="sin_sb")
            rt_sb = consts.tile([128, 128], f32, name="rt_sb")
            mask_sb = consts.tile([128, 128], f32, name="mask_sb")
            id_sb = consts.tile([128, 128], f32, name="id_sb")
            ones_f = consts.tile([128, 128], f32, name="ones_f")
            ones_sb = consts.tile([128, 128], f32, name="ones_sb")
            bq_sb = consts.tile([128, NH_C], f32, name="bq_sb")
            bk_sb = consts.tile([128, 1], f32, name="bk_sb")
            bv_sb = consts.tile([128, 1], f32, name="bv_sb")
            # constant / weight DMAs are emitted interleaved with the first
            # t-block's x-chunk DMAs (below) so the sync DMA queue delivers
            # bytes in consumption order and the PE starts within ~2 chunks.
            wq_sb = wpool.tile([128, nd, NH_C * HS], f32, name="wq_sb")
            wk_sb = wpool.tile([128, nd, HS], f32, name="wk_sb")
            wv_sb = wpool.tile([128, nd, HS], f32, name="wv_sb")
            wo_sb = wpool.tile([128, NH_C, D], f32, name="wo_sb")
            wq_re = wqd[:, :].rearrange("(n p) m -> p n m", p=128)
            wk_re = wkd[:, :].rearrange("(n p) m -> p n m", p=128)
            wv_re = wvd[:, :].rearrange("(n p) m -> p n m", p=128)

            # resident K^T [hs, t] and V [t(128-tiles), hs]
            kt_sb = resid.tile([128, t], f32, name="kt_sb")
            v_sb = resid.tile([128, nk, HS], f32, name="v_sb")

            x_re = xd[:, :].rearrange("(n p) t -> p n t", p=128)

            def emit_oproj(tb, ot_sb):
                # o_proj partial for q-block tb; emitted one block late so the
                # PE fills the next block's RoPE-latency gap with these
                # matmuls while ACT/DVE produce qf.
                ts0 = tb * 512
                for s in range(4):
                    for db in range(D // 512):
                        op_ps = bank("op_ps")
                        for h in range(NH_C):
                            nc.tensor.matmul(
                                op_ps,
                                lhsT=r(ot_sb[h][:, 128 * s : 128 * (s + 1)]),
                                rhs=r(wo_sb[:, h, 512 * db : 512 * (db + 1)]),
                                start=h == 0, stop=h == NH_C - 1,
                            )
                        ob = outp.tile([128, 512], f32, name="ob")
                        if (s + db) % 2 == 0:
                            nc.vector.tensor_copy(out=ob, in_=op_ps)
                        else:
                            nc.scalar.copy(out=ob, in_=op_ps)
                        (nc.gpsimd if gp_outdma else nc.sync).dma_start(
                            out=outd[
                                ts0 + 128 * s : ts0 + 128 * (s + 1),
                                512 * db : 512 * (db + 1),
                            ],
                            in_=ob,
                        )

            pending_oproj = None
            for tb in range(tb_n):
                ts0 = tb * 512
                # ============ phase A: projections + RoPE for this t-block
                qt_ps = [bank(f"qt_ps{h}") for h in range(NH_C)]
                kt_ps = bank("kt_ps")
                vt_ps = bank("vt_ps")
                for chunk in range(nd // 2):
                    c2 = 2 * chunk
                    xt = xin.tile([128, 2, 512], f32, name="xt")
                    nc.sync.dma_start(
                        out=r(xt),
                        in_=r(x_re[:, c2 : c2 + 2, ts0 : ts0 + 512]),
                    )
                    if tb == 0:
                        nc.sync.dma_start(
                            out=r(wq_sb[:, c2 : c2 + 2, :]),
                            in_=r(wq_re[:, c2 : c2 + 2, :]),
                        )
                        nc.sync.dma_start(
                            out=r(wk_sb[:, c2 : c2 + 2, :]),
                            in_=r(wk_re[:, c2 : c2 + 2, :]),
                        )
                        nc.sync.dma_start(
                            out=r(wv_sb[:, c2 : c2 + 2, :]),
                            in_=r(wv_re[:, c2 : c2 + 2, :]),
                        )
                        if chunk == 0:
                            nc.sync.dma_start(out=bq_sb, in_=bqd[:, :])
                            nc.sync.dma_start(out=bk_sb, in_=bkd[:, :])
                            nc.sync.dma_start(out=bv_sb, in_=bvd[:, :])
                            nc.vector.memset(ones_f, 1.0)
                            nc.scalar.copy(out=r(ones_sb), in_=ones_f)
                    for j in range(2):
                        dt = c2 + j
                        first, last = dt == 0, dt == nd - 1
                        for h in range(NH_C):
                            nc.tensor.matmul(
                                qt_ps[h],
                                lhsT=r(wq_sb[:, dt, h * HS : (h + 1) * HS]),
                                rhs=r(xt[:, j, :]),
                                start=first,
                                stop=last,
                            )
                        nc.tensor.matmul(
                            kt_ps, lhsT=r(wk_sb[:, dt, :]), rhs=r(xt[:, j, :]),
                            start=first, stop=last,
                        )
                        nc.tensor.matmul(
                            vt_ps, lhsT=r(wv_sb[:, dt, :]), rhs=r(xt[:, j, :]),
                            start=first, stop=last,
                        )

                if tb == 0:
                    # one-time loads go on the gpsimd queue so the sync queue
                    # stays dedicated to the xt stream
                    nc.gpsimd.dma_start(out=cos_sb, in_=cosd[:, :])
                    nc.gpsimd.dma_start(out=sin_sb, in_=sind[:, :])
                    nc.gpsimd.dma_start(out=r(rt_sb), in_=r(rtd[:, :]))
                    nc.gpsimd.dma_start(out=mask_sb, in_=maskd[:, :])
                    nc.gpsimd.dma_start(out=id_sb, in_=identd[:, :])
                    wo_re = wod[:, :].rearrange("(h p) m -> p h m", p=128)
                    for h in range(NH_C):
                        nc.gpsimd.dma_start(
                            out=r(wo_sb[:, h : h + 1, :]),
                            in_=r(wo_re[:, h : h + 1, :]),
                        )

                # q: bias + rope -> qf [128, h, 512]
                qf = qfp.tile([128, NH_C, 512], f32, name="qf")
                for h in range(NH_C):
                    qraw = work.tile([128, 512], f32, name="qraw")
                    nc.scalar.activation(
                        out=r(qraw), in_=qt_ps[h], func=ident_f,
                        bias=bq_sb[:, h : h + 1], scale=1.0,
                    )
                    rot_ps = bank("rot_ps")
                    nc.tensor.matmul(
                        rot_ps, lhsT=r(rt_sb[:, :]), rhs=r(qraw),
                        start=True, stop=True,
                    )
                    nc.vector.tensor_mul(
                        r(qf[:, h, :]), qraw, cos_sb[:, ts0 : ts0 + 512]
                    )
                    rtmp = work.tile([128, 512], f32, name="rtmp", bufs=2)
                    nc.vector.tensor_mul(rtmp, rot_ps, sin_sb[:, ts0 : ts0 + 512])
                    nc.vector.tensor_add(r(qf[:, h, :]), qf[:, h, :], rtmp)

                # k: bias + rope -> kt_sb slice
                kraw = work.tile([128, 512], f32, name="qraw")
                nc.scalar.activation(
                    out=r(kraw), in_=kt_ps, func=ident_f, bias=bk_sb[:, 0:1],
                    scale=1.0,
                )
                rot_ps = bank("rot_ps")
                nc.tensor.matmul(
                    rot_ps, lhsT=r(rt_sb[:, :]), rhs=r(kraw), start=True, stop=True
                )
                nc.vector.tensor_mul(
                    r(kt_sb[:, ts0 : ts0 + 512]), kraw, cos_sb[:, ts0 : ts0 + 512]
                )
                rtmp = work.tile([128, 512], f32, name="rtmp", bufs=2)
                nc.vector.tensor_mul(rtmp, rot_ps, sin_sb[:, ts0 : ts0 + 512])
                nc.vector.tensor_add(
                    r(kt_sb[:, ts0 : ts0 + 512]), kt_sb[:, ts0 : ts0 + 512], rtmp
                )

                # v: bias, then transpose to [t, hs] tiles
                vraw = work.tile([128, 512], f32, name="qraw")
                nc.scalar.activation(
                    out=vraw, in_=vt_ps, func=ident_f, bias=bv_sb[:, 0:1], scale=1.0
                )
                for s in range(4):
                    vt_tp = bank("vt_tp")
                    nc.tensor.transpose(
                        vt_tp[:, 0:128], vraw[:, 128 * s : 128 * (s + 1)], id_sb[:, :]
                    )
                    nc.scalar.copy(
                        out=r(v_sb[:, 4 * tb + s, :]), in_=vt_tp[:, 0:128]
                    )

                # previous block's o_proj: PE chews on it while ACT/DVE
                # finish this block's RoPE chain (qf not needed yet).
                if pending_oproj is not None:
                    emit_oproj(tb - 1, pending_oproj)

                # ============ phase B: attention for q-block jq == tb
                # heads in pairs: fp32r matmul outputs must sit at PSUM
                # col-group 0, so each head's denominator needs its own bank.
                # Software-pipelined by one k-tile: PV/denom of tile i-1 issue
                # while ACT computes exp of tile i, so the PE never waits on
                # the st->exp->mask chain.
                ot_sb = {}
                imax = 4 * tb + 3
                for hp in range(NH_C // 2):
                    heads = (2 * hp, 2 * hp + 1)
                    ot_ps = {h: bank(f"ot_ps{h}") for h in heads}
                    den_ps = {h: bank(f"den_ps{h}") for h in heads}

                    def emit_pv_den(i, pts, c0):
                        first, last = i == 0, i == imax
                        for h in heads:
                            nc.tensor.matmul(
                                ot_ps[h][:, c0:],
                                lhsT=r(v_sb[:, i, :]),
                                rhs=r(pts[h][:, c0:]),
                                start=first, stop=last,
                            )
                            nc.tensor.matmul(
                                den_ps[h][0:1, c0:],
                                lhsT=r(ones_sb[:, 0:1]),
                                rhs=r(pts[h][:, c0:]),
                                start=first, stop=last,
                                skip_group_check=True,
                            )

                    prev = None
                    for i in range(imax + 1):
                        c0 = 128 * max(0, i - 4 * tb)
                        diag = i >= 4 * tb
                        pts = {}
                        for h in heads:
                            st_ps = bank("st_ps")
                            nc.tensor.matmul(
                                st_ps[:, c0:],
                                lhsT=r(kt_sb[:, 128 * i : 128 * (i + 1)]),
                                rhs=r(qf[:, h, c0:]),
                                start=True, stop=True,
                            )
                            pt = work.tile([128, 512], f32, name="pt", bufs=4)
                            nc.scalar.activation(
                                out=r(pt[:, c0:]), in_=st_ps[:, c0:], func=exp_f,
                                scale=INV_SQRT_HS,
                            )
                            if diag:
                                if gp_mask:
                                    # zero strictly-lower triangle (tk > tq)
                                    # on the otherwise-idle GpSimd engine
                                    nc.gpsimd.affine_select(
                                        out=r(pt[:, c0 : c0 + 128]),
                                        in_=pt[:, c0 : c0 + 128],
                                        compare_op=mybir.AluOpType.is_ge,
                                        fill=0.0,
                                        base=0,
                                        pattern=[[1, 128]],
                                        channel_multiplier=-1,
                                    )
                                else:
                                    nc.vector.tensor_mul(
                                        r(pt[:, c0 : c0 + 128]),
                                        pt[:, c0 : c0 + 128],
                                        mask_sb,
                                    )
                            pts[h] = pt
                        if prev is not None:
                            emit_pv_den(*prev)
                        prev = (i, pts, c0)
                    emit_pv_den(*prev)

                    # normalize each head's O^T by its softmax denominator.
                    # The denominator rows bounce through DRAM: scatter across
                    # 128 partitions (one cheap [128,8] reciprocal per pair
                    # instead of 3.3us single-lane [1,512] ones), then a
                    # step-0 broadcast DMA from DRAM fans the reciprocal out
                    # to [128,512] -- no PE or PSUM involvement at all.
                    # Evacuate O^T and the denominator rows immediately (ACT
                    # copies) so all PSUM banks free before the next phase's
                    # matmuls.  Normalization then runs with ~a full t-block
                    # of slack: broadcast the RAW denominator across
                    # partitions with a K=1 ones matmul (dedicated PSUM tag),
                    # and divide on the otherwise-idle GpSimd engine -- no
                    # DVE head-of-line blocking, no DRAM round trips.
                    osb_h = {}
                    denrow = {}
                    for h in heads:
                        osb = otp.tile([128, 512], f32, name="osb", bufs=8)
                        nc.scalar.copy(out=r(osb), in_=ot_ps[h])
                        osb_h[h] = osb
                        dr = work.tile([1, 512], f32, name="den_sb", bufs=2)
                        nc.scalar.copy(out=r(dr), in_=den_ps[h][0:1, :])
                        denrow[h] = dr
                    for h in heads:
                        # broadcast the RAW denominator right away (PE never
                        # waits on the reciprocal), then 1/x on DVE with slack
                        bc_ps = psum.tile(
                            [128, 512], f32, tag="bc", name="bc_ps", bufs=1
                        )
                        nc.tensor.matmul(
                            bc_ps,
                            lhsT=r(ones_sb[0:1, 0:128]),
                            rhs=r(denrow[h]),
                            start=True, stop=True,
                        )
                        bc_sb = work.tile([128, 512], f32, name="bc_sb", bufs=2)
                        nc.scalar.copy(out=bc_sb, in_=bc_ps)
                        nc.vector.reciprocal(out=bc_sb, in_=bc_sb)
                        nc.vector.tensor_mul(r(osb_h[h]), osb_h[h], bc_sb)
                        ot_sb[h] = osb_h[h]

                pending_oproj = ot_sb

            emit_oproj(tb_n - 1, pending_oproj)

    nc.compile()
    return nc


def shard_inputs(x, cos, sin, Wq, bq, Wkv, bkv, Wo, t=T):
    """Build the 8 per-core input maps (core c -> batch c//4, group c%4)."""
    f32 = np.float32
    hs = HS
    rot = np.zeros((hs, hs), f32)
    for i in range(hs // 2):
        rot[i, i + hs // 2] = -1.0
        rot[i + hs // 2, i] = 1.0
    r_t = np.ascontiguousarray(rot.T)
    mask_ut = np.triu(np.ones((128, 128), f32))
    ident = np.eye(128, dtype=f32)
    cos_t = np.ascontiguousarray(cos.T.astype(f32))
    sin_t = np.ascontiguousarray(sin.T.astype(f32))

    xts = [np.ascontiguousarray(x[b].T.astype(f32)) for b in range(x.shape[0])]
    per_g = []
    for g in range(4):
        per_g.append(
            dict(
                wq_t=np.ascontiguousarray(Wq[512 * g : 512 * g + 512].T.astype(f32)),
                b_q=np.ascontiguousarray(
                    bq[512 * g : 512 * g + 512].reshape(4, 128).T.astype(f32)
                ),
                wk_t=np.ascontiguousarray(
                    Wkv[128 * g : 128 * g + 128].T.astype(f32)
                ),
                b_k=np.ascontiguousarray(
                    bkv[128 * g : 128 * g + 128].reshape(128, 1).astype(f32)
                ),
                wv_t=np.ascontiguousarray(
                    Wkv[512 + 128 * g : 512 + 128 * g + 128].T.astype(f32)
                ),
                b_v=np.ascontiguousarray(
                    bkv[512 + 128 * g : 512 + 128 * g + 128]
                    .reshape(128, 1)
                    .astype(f32)
                ),
                wo_t=np.ascontiguousarray(
                    Wo[:, 512 * g : 512 * g + 512].T.astype(f32)
                ),
            )
        )

    in_maps = []
    for c in range(4 * x.shape[0]):
        b, g = c // 4, c % 4
        m = dict(per_g[g])
        m.update(
            x_t=xts[b], cos_t=cos_t, sin_t=sin_t,
            r_t=r_t, mask_ut=mask_ut, ident=ident,
        )
        in_maps.append(m)
    return in_maps


def run_on_hw(in_maps, t=T, trace=False, mm_r=True, **flags):
    from concourse.bass_utils import run_bass_kernel_spmd

    key = (t, mm_r, tuple(sorted(flags.items())))
    if key not in _NC_CACHE:
        _NC_CACHE[key] = build_nc(t, mm_r=mm_r, **flags)
    nc = _NC_CACHE[key]
    res = run_bass_kernel_spmd(
        nc, in_maps, core_ids=list(range(len(in_maps))), trace=trace
    )
    return res


def kernel(x, cos, sin, Wq, bq, Wkv, bkv, Wo):
    x = np.asarray(x)
    in_maps = shard_inputs(
        x, np.asarray(cos), np.asarray(sin), np.asarray(Wq), np.asarray(bq),
        np.asarray(Wkv), np.asarray(bkv), np.asarray(Wo),
    )
    res = run_on_hw(in_maps, t=T, trace=False)
    out = np.zeros((B, T, D), np.float32)
    for c, rmap in enumerate(res.results):
        out[c // 4] += rmap["out"]
    return out



# revision 5
# speedup vs baseline: 1.2760x; 1.2760x over previous
"""Causal group-query attention on 8 Trainium2 NeuronCores.

Sharding: core c -> (batch b = c // 4, kv-group g = c % 4).
Each core owns batch element b, q-heads [4g, 4g+4) and kv-group g (n_rep = 4,
so those 4 q-heads attend to exactly kv-group g's k/v).  Every core computes
its partial o_proj output (contracting head-concat columns [512g, 512g+512)),
and the host sums the 4 partials per batch element (the "all-reduce after
o_proj" done host-side since we return full outputs anyway).

All matmul operands are fp16 (e5m10): same 1 cycle/row PE rate as fp32r at
N>=256 but no 4x penalty at small N, half the SBUF/DMA footprint, FWL weight
loads, and 1024-wide moving operands.  fp16 quantization (~0.03% RMS) keeps
the end-to-end rel err ~1e-3, far under the 2e-2 gate.  PSUM accumulation is
fp32 throughout.

Per-core kernel (T=2048, D=2048, HS=128):
  phase A (per 512-wide t-block): stream x^T fp16, q projections in two
    head-pair passes (each into one 2-bank PSUM tile), RoPE per head on
    ACT/DVE with the rotate-half permutation as a 128x128 matmul; then k/v
    projections (shared 2-bank tile), k-RoPE, v bias + transpose to [t, hs]
    fp16 tiles.  The previous block's o_proj is emitted after the k/v pass so
    the PE chews on it while ACT/DVE finish the RoPE chains.
  phase B: heads processed in pairs with merged matmuls: for each causally
    valid 128-wide k-tile i, ONE S matmul per pair (moving qf [128, 2, N'],
    output S^T pair tile [128, 2, 512] = 2 PSUM banks), ONE exp ACTIVATE over
    both heads, triangular-mask multiplies on diagonal subtiles (DVE), then
    ONE PV matmul (moving pt pair) and two 1-row denominator matmuls per
    tile, all PSUM-accumulated.  Software-pipelined by one k-tile so the PE
    never waits on the S->exp chain.
  normalize: denominator rows evacuate via ACT, broadcast across partitions
    with a K=1 ones matmul, reciprocal_approx_fast on DVE (~5x faster than
    exact reciprocal; 18-bit accuracy), multiply into the evacuated O^T pair.
  phase C: o_proj out[tq, d] = sum_h O^T_h-stationary @ Wo^T_h moving, fp16
    out tiles DMA'd to DRAM; host upcasts to f32 and sums the 4 partials.

PSUM plan (8 banks): tag "pb" = 3 bufs of [128, 2, 512] f32 (6 banks) rotating
through qt-pair x2 / ktvt / S-pair x2 / O-pair; tag "b1" = 2 bufs of
[128, 512] f32 (2 banks) for rot/vtransp/oproj/denominator/broadcast.
"""

import math

import numpy as np

B, T, D = 2, 2048, 2048
N_HEAD, N_GROUP = 16, 4
HS = D // N_HEAD  # 128
N_REP = N_HEAD // N_GROUP  # 4
NH_C = N_HEAD // N_GROUP  # heads per core = 4
INV_SQRT_HS = 1.0 / math.sqrt(HS)

_NC_CACHE: dict = {}


def build_nc(t=T, gp_outdma=True):
    """Build and compile the per-core Bass program. Returns the compiled nc."""
    import concourse.bass as bass  # noqa: F401
    import concourse.mybir as mybir
    import concourse.tile as tile
    from concourse import bacc

    f32 = mybir.dt.float32
    f16 = mybir.dt.float16
    ident_f = mybir.ActivationFunctionType.Identity
    exp_f = mybir.ActivationFunctionType.Exp

    nd = D // 128  # d-tiles (contraction) = 16
    tb_n = t // 512  # 512-wide t blocks
    nk = t // 128  # 128-wide k tiles

    nc = bacc.Bacc("TRN2", target_bir_lowering=False, debug=False)

    xd = nc.dram_tensor("x_t", [D, t], f16, kind="ExternalInput")
    wqd = nc.dram_tensor("wq_t", [D, NH_C * HS], f16, kind="ExternalInput")
    wkd = nc.dram_tensor("wk_t", [D, HS], f16, kind="ExternalInput")
    wvd = nc.dram_tensor("wv_t", [D, HS], f16, kind="ExternalInput")
    wod = nc.dram_tensor("wo_t", [NH_C * HS, D], f16, kind="ExternalInput")
    cosd = nc.dram_tensor("cos_t", [HS, t], f16, kind="ExternalInput")
    sind = nc.dram_tensor("sin_t", [HS, t], f16, kind="ExternalInput")
    bqd = nc.dram_tensor("b_q", [HS, NH_C], f32, kind="ExternalInput")
    bkd = nc.dram_tensor("b_k", [HS, 1], f32, kind="ExternalInput")
    bvd = nc.dram_tensor("b_v", [HS, 1], f32, kind="ExternalInput")
    rtd = nc.dram_tensor("r_t", [HS, HS], f16, kind="ExternalInput")
    maskd = nc.dram_tensor("mask_ut", [128, 128], f16, kind="ExternalInput")
    identd = nc.dram_tensor("ident", [128, 128], f16, kind="ExternalInput")
    outd = nc.dram_tensor("out", [t, D], f16, kind="ExternalOutput")

    with tile.TileContext(nc) as tc:
        with (
            tc.tile_pool(name="consts", bufs=1) as consts,
            tc.tile_pool(name="wpool", bufs=1) as wpool,
            tc.tile_pool(name="resid", bufs=1) as resid,
            tc.tile_pool(name="xin", bufs=10) as xin,
            tc.tile_pool(name="work", bufs=3) as work,
            tc.tile_pool(name="ptp", bufs=4) as ptp,
            tc.tile_pool(name="qfp", bufs=2) as qfp,
            tc.tile_pool(name="otp", bufs=4) as otp,
            tc.tile_pool(name="outp", bufs=4) as outp,
            tc.tile_pool(name="psum", bufs=3, space="PSUM") as psum,
        ):
            def pb(name):
                return psum.tile([128, 2, 512], f32, tag="pb", bufs=3, name=name)

            def b1(name):
                return psum.tile([128, 512], f32, tag="b1", bufs=2, name=name)

            # ---- constants / weights (loaded once) ----
            cos_sb = consts.tile([128, t], f16, name="cos_sb")
            sin_sb = consts.tile([128, t], f16, name="sin_sb")
            rt_sb = consts.tile([128, 128], f16, name="rt_sb")
            mask_sb = consts.tile([128, 128], f16, name="mask_sb")
            id_sb = consts.tile([128, 128], f16, name="id_sb")
            ones_f = consts.tile([128, 128], f32, name="ones_f")
            ones_sb = consts.tile([128, 128], f16, name="ones_sb")
            bq_sb = consts.tile([128, NH_C], f32, name="bq_sb")
            bk_sb = consts.tile([128, 1], f32, name="bk_sb")
            bv_sb = consts.tile([128, 1], f32, name="bv_sb")
            wq_sb = wpool.tile([128, nd, NH_C * HS], f16, name="wq_sb")
            wk_sb = wpool.tile([128, nd, HS], f16, name="wk_sb")
            wv_sb = wpool.tile([128, nd, HS], f16, name="wv_sb")
            wo_sb = wpool.tile([128, NH_C, D], f16, name="wo_sb")
            wq_re = wqd[:, :].rearrange("(n p) m -> p n m", p=128)
            wk_re = wkd[:, :].rearrange("(n p) m -> p n m", p=128)
            wv_re = wvd[:, :].rearrange("(n p) m -> p n m", p=128)

            # resident K^T [hs, t] and V [t(128-tiles), hs]
            kt_sb = resid.tile([128, t], f16, name="kt_sb")
            v_sb = resid.tile([128, nk, HS], f16, name="v_sb")

            x_re = xd[:, :].rearrange("(n p) t -> p n t", p=128)

            def emit_oproj(tb, ot_sb):
                # o_proj partial for q-block tb; emitted one block late so the
                # PE fills the RoPE-latency gap with these matmuls.
                ts0 = tb * 512
                for s in range(4):
                    for db in range(D // 512):
                        op_ps = b1("op_ps")
                        for h in range(NH_C):
                            nc.tensor.matmul(
                                op_ps,
                                lhsT=ot_sb[h // 2][:, h % 2, 128 * s : 128 * (s + 1)],
                                rhs=wo_sb[:, h, 512 * db : 512 * (db + 1)],
                                start=h == 0, stop=h == NH_C - 1,
                            )
                        ob = outp.tile([128, 512], f16, name="ob")
                        if (s + db) % 2 == 0:
                            nc.vector.tensor_copy(out=ob, in_=op_ps)
                        else:
                            nc.scalar.copy(out=ob, in_=op_ps)
                        (nc.gpsimd if gp_outdma else nc.sync).dma_start(
                            out=outd[
                                ts0 + 128 * s : ts0 + 128 * (s + 1),
                                512 * db : 512 * (db + 1),
                            ],
                            in_=ob,
                        )

            def rope(dst, raw, rot_ps, ts0):
                # dst = raw * cos + (R @ raw) * sin   (rot_ps already in PSUM)
                nc.vector.tensor_mul(dst, raw, cos_sb[:, ts0 : ts0 + 512])
                rtmp = work.tile([128, 512], f16, name="rtmp", bufs=2)
                nc.vector.tensor_mul(rtmp, rot_ps, sin_sb[:, ts0 : ts0 + 512])
                nc.vector.tensor_add(dst, dst, rtmp)

            pending_oproj = None
            for tb in range(tb_n):
                ts0 = tb * 512
                # ============ phase A: projections + RoPE for this t-block
                # q heads in two pair-passes, then k/v, so at most two 2-bank
                # PSUM tiles are alive at once.
                xts = []
                for chunk in range(nd // 2):
                    c2 = 2 * chunk
                    xt = xin.tile([128, 2, 512], f16, name="xt")
                    nc.sync.dma_start(
                        out=xt, in_=x_re[:, c2 : c2 + 2, ts0 : ts0 + 512]
                    )
                    xts.append(xt)
                    if tb == 0:
                        nc.sync.dma_start(
                            out=wq_sb[:, c2 : c2 + 2, :], in_=wq_re[:, c2 : c2 + 2, :]
                        )
                        nc.sync.dma_start(
                            out=wk_sb[:, c2 : c2 + 2, :], in_=wk_re[:, c2 : c2 + 2, :]
                        )
                        nc.sync.dma_start(
                            out=wv_sb[:, c2 : c2 + 2, :], in_=wv_re[:, c2 : c2 + 2, :]
                        )
                        if chunk == 0:
                            nc.sync.dma_start(out=bq_sb, in_=bqd[:, :])
                            nc.sync.dma_start(out=bk_sb, in_=bkd[:, :])
                            nc.sync.dma_start(out=bv_sb, in_=bvd[:, :])
                            nc.vector.memset(ones_f, 1.0)
                            nc.scalar.copy(out=ones_sb, in_=ones_f)
                            # one-time loads on the gpsimd queue so the sync
                            # queue stays dedicated to the xt/w stream
                            nc.gpsimd.dma_start(out=cos_sb, in_=cosd[:, :])
                            nc.gpsimd.dma_start(out=sin_sb, in_=sind[:, :])
                            nc.gpsimd.dma_start(out=rt_sb, in_=rtd[:, :])
                            nc.gpsimd.dma_start(out=mask_sb, in_=maskd[:, :])
                            nc.gpsimd.dma_start(out=id_sb, in_=identd[:, :])
                            wo_re = wod[:, :].rearrange("(h p) m -> p h m", p=128)
                            for h in range(NH_C):
                                nc.gpsimd.dma_start(
                                    out=wo_sb[:, h : h + 1, :],
                                    in_=wo_re[:, h : h + 1, :],
                                )

                qf = qfp.tile([128, NH_C, 512], f16, name="qf")
                for p in range(2):
                    qt2 = pb(f"qt2_{p}")
                    for chunk in range(nd // 2):
                        for j in range(2):
                            dt = 2 * chunk + j
                            first, last = dt == 0, dt == nd - 1
                            for hh in range(2):
                                h = 2 * p + hh
                                nc.tensor.matmul(
                                    qt2[:, hh, :],
                                    lhsT=wq_sb[:, dt, h * HS : (h + 1) * HS],
                                    rhs=xts[chunk][:, j, :],
                                    start=first, stop=last,
                                    skip_group_check=True,
                                )
                    # bias + rope for this pair
                    for hh in range(2):
                        h = 2 * p + hh
                        qraw = work.tile([128, 512], f16, name="qraw")
                        nc.scalar.activation(
                            out=qraw, in_=qt2[:, hh, :], func=ident_f,
                            bias=bq_sb[:, h : h + 1], scale=1.0,
                        )
                        rot_ps = b1("rot_ps")
                        nc.tensor.matmul(
                            rot_ps, lhsT=rt_sb, rhs=qraw, start=True, stop=True
                        )
                        rope(qf[:, h, :], qraw, rot_ps, ts0)

                # k/v projections (shared 2-bank tile: k half 0, v half 1)
                ktvt = pb("ktvt")
                for chunk in range(nd // 2):
                    for j in range(2):
                        dt = 2 * chunk + j
                        first, last = dt == 0, dt == nd - 1
                        nc.tensor.matmul(
                            ktvt[:, 0, :], lhsT=wk_sb[:, dt, :],
                            rhs=xts[chunk][:, j, :],
                            start=first, stop=last, skip_group_check=True,
                        )
                        nc.tensor.matmul(
                            ktvt[:, 1, :], lhsT=wv_sb[:, dt, :],
                            rhs=xts[chunk][:, j, :],
                            start=first, stop=last, skip_group_check=True,
                        )

                kraw = work.tile([128, 512], f16, name="qraw")
                nc.scalar.activation(
                    out=kraw, in_=ktvt[:, 0, :], func=ident_f,
                    bias=bk_sb[:, 0:1], scale=1.0,
                )
                rot_ps = b1("rot_ps")
                nc.tensor.matmul(rot_ps, lhsT=rt_sb, rhs=kraw, start=True, stop=True)
                rope(kt_sb[:, ts0 : ts0 + 512], kraw, rot_ps, ts0)

                # v: bias (fp16 cast), then transpose to [t, hs] tiles
                vraw = work.tile([128, 512], f16, name="qraw")
                nc.scalar.activation(
                    out=vraw, in_=ktvt[:, 1, :], func=ident_f,
                    bias=bv_sb[:, 0:1], scale=1.0,
                )
                for s in range(4):
                    vt_tp = b1("vt_tp")
                    vt16 = vt_tp[:, 0:64].bitcast(f16)
                    nc.tensor.transpose(
                        vt16, vraw[:, 128 * s : 128 * (s + 1)], id_sb
                    )
                    nc.scalar.copy(out=v_sb[:, 4 * tb + s, :], in_=vt16)

                # previous block's o_proj: PE chews on it while ACT/DVE
                # finish this block's RoPE chain (qf not needed yet).
                if pending_oproj is not None:
                    emit_oproj(tb - 1, pending_oproj)

                # ============ phase B: attention for q-block jq == tb
                # Pair-merged: one S matmul / exp / PV matmul per head-pair
                # and k-tile.  Software-pipelined by one k-tile.
                ot_sb = {}
                imax = 4 * tb + 3
                for p in range(2):
                    ot2 = pb(f"ot2_{p}")
                    den = b1(f"den_{p}")

                    def emit_pv_den(i, pt, c0, ot2=ot2, den=den):
                        first, last = i == 0, i == imax
                        for hh in range(2):
                            nc.tensor.matmul(
                                ot2[:, hh, c0:], lhsT=v_sb[:, i, :],
                                rhs=pt[:, hh, c0:],
                                start=first, stop=last, skip_group_check=True,
                            )
                        for hh in range(2):
                            nc.tensor.matmul(
                                den[32 * hh : 32 * hh + 1, c0:],
                                lhsT=ones_sb[:, 0:1],
                                rhs=pt[:, hh, c0:],
                                start=first, stop=last,
                                skip_group_check=True,
                            )

                    prev = None
                    for i in range(imax + 1):
                        c0 = 128 * max(0, i - 4 * tb)
                        st2 = pb("st2")
                        for hh in range(2):
                            nc.tensor.matmul(
                                st2[:, hh, c0:],
                                lhsT=kt_sb[:, 128 * i : 128 * (i + 1)],
                                rhs=qf[:, 2 * p + hh, c0:],
                                start=True, stop=True, skip_group_check=True,
                            )
                        pt = ptp.tile([128, 2, 512], f16, name="pt")
                        nc.scalar.activation(
                            out=pt[:, :, c0:], in_=st2[:, :, c0:], func=exp_f,
                            scale=INV_SQRT_HS,
                        )
                        if i >= 4 * tb:
                            for hh in range(2):
                                nc.vector.tensor_mul(
                                    pt[:, hh, c0 : c0 + 128],
                                    pt[:, hh, c0 : c0 + 128],
                                    mask_sb,
                                )
                        if prev is not None:
                            emit_pv_den(*prev)
                        prev = (i, pt, c0)
                    emit_pv_den(*prev)

                    # evacuate O^T pair immediately (frees PSUM), then
                    # normalize in SBUF with slack: broadcast the raw
                    # denominator with a K=1 ones matmul, fast-approx
                    # reciprocal on DVE, multiply in place.
                    osb = otp.tile([128, 2, 512], f16, name="osb")
                    nc.scalar.copy(out=osb, in_=ot2)
                    denrow = work.tile([33, 512], f16, name="denrow", bufs=2)
                    nc.scalar.copy(out=denrow[0:1, :], in_=den[0:1, :])
                    nc.scalar.copy(out=denrow[32:33, :], in_=den[32:33, :])
                    for hh in range(2):
                        bc_ps = b1("bc_ps")
                        nc.tensor.matmul(
                            bc_ps,
                            lhsT=ones_sb[32 * hh : 32 * hh + 1, 0:128],
                            rhs=denrow[32 * hh : 32 * hh + 1, :],
                            start=True, stop=True,
                        )
                        bcr = work.tile([128, 512], f32, name="bcr", bufs=2)
                        nc.vector.reciprocal_approx_fast(out=bcr, in_=bc_ps)
                        nc.vector.tensor_mul(osb[:, hh, :], osb[:, hh, :], bcr)
                    ot_sb[p] = osb

                pending_oproj = ot_sb

            emit_oproj(tb_n - 1, pending_oproj)

    nc.compile()
    return nc


def shard_inputs(x, cos, sin, Wq, bq, Wkv, bkv, Wo, t=T):
    """Build the 8 per-core input maps (core c -> batch c//4, group c%4)."""
    f16 = np.float16
    f32 = np.float32
    hs = HS
    rot = np.zeros((hs, hs), f32)
    for i in range(hs // 2):
        rot[i, i + hs // 2] = -1.0
        rot[i + hs // 2, i] = 1.0
    r_t = np.ascontiguousarray(rot.T.astype(f16))
    mask_ut = np.triu(np.ones((128, 128), f16))
    ident = np.eye(128, dtype=f16)
    cos_t = np.ascontiguousarray(np.asarray(cos, f32).T.astype(f16))
    sin_t = np.ascontiguousarray(np.asarray(sin, f32).T.astype(f16))

    xts = [
        np.ascontiguousarray(np.asarray(x[b], f32).T.astype(f16))
        for b in range(x.shape[0])
    ]
    per_g = []
    for g in range(4):
        per_g.append(
            dict(
                wq_t=np.ascontiguousarray(
                    Wq[512 * g : 512 * g + 512].T.astype(f16)
                ),
                b_q=np.ascontiguousarray(
                    bq[512 * g : 512 * g + 512].reshape(4, 128).T.astype(f32)
                ),
                wk_t=np.ascontiguousarray(
                    Wkv[128 * g : 128 * g + 128].T.astype(f16)
                ),
                b_k=np.ascontiguousarray(
                    bkv[128 * g : 128 * g + 128].reshape(128, 1).astype(f32)
                ),
                wv_t=np.ascontiguousarray(
                    Wkv[512 + 128 * g : 512 + 128 * g + 128].T.astype(f16)
                ),
                b_v=np.ascontiguousarray(
                    bkv[512 + 128 * g : 512 + 128 * g + 128]
                    .reshape(128, 1)
                    .astype(f32)
                ),
                wo_t=np.ascontiguousarray(
                    Wo[:, 512 * g : 512 * g + 512].T.astype(f16)
                ),
            )
        )

    in_maps = []
    for c in range(4 * x.shape[0]):
        b, g = c // 4, c % 4
        m = dict(per_g[g])
        m.update(
            x_t=xts[b], cos_t=cos_t, sin_t=sin_t,
            r_t=r_t, mask_ut=mask_ut, ident=ident,
        )
        in_maps.append(m)
    return in_maps


def run_on_hw(in_maps, t=T, trace=False, **flags):
    from concourse.bass_utils import run_bass_kernel_spmd

    key = (t, tuple(sorted(flags.items())))
    if key not in _NC_CACHE:
        _NC_CACHE[key] = build_nc(t, **flags)
    nc = _NC_CACHE[key]
    res = run_bass_kernel_spmd(
        nc, in_maps, core_ids=list(range(len(in_maps))), trace=trace
    )
    return res


def kernel(x, cos, sin, Wq, bq, Wkv, bkv, Wo):
    x = np.asarray(x)
    in_maps = shard_inputs(
        x, np.asarray(cos), np.asarray(sin), np.asarray(Wq), np.asarray(bq),
        np.asarray(Wkv), np.asarray(bkv), np.asarray(Wo),
    )
    res = run_on_hw(in_maps, t=T, trace=False)
    out = np.zeros((B, T, D), np.float32)
    for c, rmap in enumerate(res.results):
        out[c // 4] += rmap["out"].astype(np.float32)
    return out


# revision 9
# speedup vs baseline: 1.3489x; 1.0572x over previous
"""Causal group-query attention on 8 Trainium2 NeuronCores.

Sharding: core c -> (batch b = c // 4, kv-group g = c % 4).
Each core owns batch element b, q-heads [4g, 4g+4) and kv-group g (n_rep = 4,
so those 4 q-heads attend to exactly kv-group g's k/v).  Every core computes
its partial o_proj output (contracting head-concat columns [512g, 512g+512)),
and the host sums the 4 partials per batch element (the "all-reduce after
o_proj" done host-side since we return full outputs anyway).

All matmul operands are fp16 (e5m10): same 1 cycle/row PE rate as fp32r at
N>=256 but no 4x penalty at small N, half the SBUF/DMA footprint, FWL weight
loads, and 1024-wide moving operands.  fp16 quantization (~0.03% RMS) keeps
the end-to-end rel err ~1e-3, far under the 2e-2 gate.  PSUM accumulation is
fp32 throughout.

Per-core kernel (T=2048, D=2048, HS=128):
  phase A (per 512-wide t-block): stream x^T fp16, q projections in two
    head-pair passes (each into one 2-bank PSUM tile), RoPE per head on
    ACT/DVE with the rotate-half permutation as a 128x128 matmul; then k/v
    projections (shared 2-bank tile), k-RoPE, v bias + transpose to [t, hs]
    fp16 tiles.  The previous block's o_proj is emitted after the k/v pass so
    the PE chews on it while ACT/DVE finish the RoPE chains.
  phase B: heads processed in pairs with merged matmuls: for each causally
    valid 128-wide k-tile i, ONE S matmul per pair (moving qf [128, 2, N'],
    output S^T pair tile [128, 2, 512] = 2 PSUM banks), ONE exp ACTIVATE over
    both heads, triangular-mask multiplies on diagonal subtiles (DVE), then
    ONE PV matmul (moving pt pair) and two 1-row denominator matmuls per
    tile, all PSUM-accumulated.  Software-pipelined by one k-tile so the PE
    never waits on the S->exp chain.
  normalize: denominator rows evacuate via ACT, broadcast across partitions
    with a K=1 ones matmul, reciprocal_approx_fast on DVE (~5x faster than
    exact reciprocal; 18-bit accuracy), multiply into the evacuated O^T pair.
  phase C: o_proj out[tq, d] = sum_h O^T_h-stationary @ Wo^T_h moving, fp16
    out tiles DMA'd to DRAM; host upcasts to f32 and sums the 4 partials.

PSUM plan (8 banks): tag "pb" = 3 bufs of [128, 2, 512] f32 (6 banks) rotating
through qt-pair x2 / ktvt / S-pair x2 / O-pair; tag "b1" = 2 bufs of
[128, 512] f32 (2 banks) for rot/vtransp/oproj/denominator/broadcast.
"""

import math

import numpy as np

B, T, D = 2, 2048, 2048
N_HEAD, N_GROUP = 16, 4
HS = D // N_HEAD  # 128
N_REP = N_HEAD // N_GROUP  # 4
NH_C = N_HEAD // N_GROUP  # heads per core = 4
INV_SQRT_HS = 1.0 / math.sqrt(HS)

_NC_CACHE: dict = {}


def build_nc(t=T, gp_outdma=True, dve_den=True):
    """Build and compile the per-core Bass program. Returns the compiled nc."""
    import concourse.bass as bass  # noqa: F401
    import concourse.mybir as mybir
    import concourse.tile as tile
    from concourse import bacc

    f32 = mybir.dt.float32
    f16 = mybir.dt.float16
    ident_f = mybir.ActivationFunctionType.Identity
    exp_f = mybir.ActivationFunctionType.Exp

    nd = D // 128  # d-tiles (contraction) = 16
    tb_n = t // 512  # 512-wide t blocks
    nk = t // 128  # 128-wide k tiles

    nc = bacc.Bacc("TRN2", target_bir_lowering=False, debug=False)

    xd = nc.dram_tensor("x_t", [D, t], f16, kind="ExternalInput")
    wqd = nc.dram_tensor("wq_t", [D, NH_C * HS], f16, kind="ExternalInput")
    wkd = nc.dram_tensor("wk_t", [D, HS], f16, kind="ExternalInput")
    wvd = nc.dram_tensor("wv_t", [D, HS], f16, kind="ExternalInput")
    wod = nc.dram_tensor("wo_t", [NH_C * HS, D], f16, kind="ExternalInput")
    cosd = nc.dram_tensor("cos_t", [HS, t], f16, kind="ExternalInput")
    sind = nc.dram_tensor("sin_t", [HS, t], f16, kind="ExternalInput")
    bqd = nc.dram_tensor("b_q", [HS, NH_C], f32, kind="ExternalInput")
    bkd = nc.dram_tensor("b_k", [HS, 1], f32, kind="ExternalInput")
    bvd = nc.dram_tensor("b_v", [HS, 1], f32, kind="ExternalInput")
    rtd = nc.dram_tensor("r_t", [HS, HS], f16, kind="ExternalInput")
    maskd = nc.dram_tensor("mask_ut", [128, 128], f16, kind="ExternalInput")
    identd = nc.dram_tensor("ident", [128, 128], f16, kind="ExternalInput")
    outd = nc.dram_tensor("out", [t, D], f16, kind="ExternalOutput")

    with tile.TileContext(nc) as tc:
        with (
            tc.tile_pool(name="consts", bufs=1) as consts,
            tc.tile_pool(name="wpool", bufs=1) as wpool,
            tc.tile_pool(name="resid", bufs=1) as resid,
            tc.tile_pool(name="xin", bufs=10) as xin,
            tc.tile_pool(name="work", bufs=3) as work,
            tc.tile_pool(name="ptp", bufs=4) as ptp,
            tc.tile_pool(name="qfp", bufs=2) as qfp,
            tc.tile_pool(name="otp", bufs=4) as otp,
            tc.tile_pool(name="outp", bufs=4) as outp,
            tc.tile_pool(name="psum", bufs=3, space="PSUM") as psum,
        ):
            def pb(name):
                return psum.tile([128, 2, 512], f32, tag="pb", bufs=3, name=name)

            def b1(name):
                return psum.tile([128, 512], f32, tag="b1", bufs=2, name=name)

            # ---- constants / weights (loaded once) ----
            cos_sb = consts.tile([128, t], f16, name="cos_sb")
            sin_sb = consts.tile([128, t], f16, name="sin_sb")
            rt_sb = consts.tile([128, 128], f16, name="rt_sb")
            mask_sb = consts.tile([128, 128], f16, name="mask_sb")
            id_sb = consts.tile([128, 128], f16, name="id_sb")
            ones_f = consts.tile([128, 128], f32, name="ones_f")
            ones_sb = consts.tile([128, 128], f16, name="ones_sb")
            bq_sb = consts.tile([128, NH_C], f32, name="bq_sb")
            bk_sb = consts.tile([128, 1], f32, name="bk_sb")
            bv_sb = consts.tile([128, 1], f32, name="bv_sb")
            wq_sb = wpool.tile([128, nd, NH_C * HS], f16, name="wq_sb")
            wk_sb = wpool.tile([128, nd, HS], f16, name="wk_sb")
            wv_sb = wpool.tile([128, nd, HS], f16, name="wv_sb")
            wo_sb = wpool.tile([128, NH_C, D], f16, name="wo_sb")
            wq_re = wqd[:, :].rearrange("(n p) m -> p n m", p=128)
            wk_re = wkd[:, :].rearrange("(n p) m -> p n m", p=128)
            wv_re = wvd[:, :].rearrange("(n p) m -> p n m", p=128)

            # resident K^T [hs, t] and V [t(128-tiles), hs]
            kt_sb = resid.tile([128, t], f16, name="kt_sb")
            v_sb = resid.tile([128, nk, HS], f16, name="v_sb")

            x_re = xd[:, :].rearrange("(n p) t -> p n t", p=128)

            def emit_oproj(tb, ot_sb):
                # o_proj partial for q-block tb; emitted one block late so the
                # PE fills the RoPE-latency gap with these matmuls.
                ts0 = tb * 512
                for s in range(4):
                    for db in range(D // 512):
                        op_ps = b1("op_ps")
                        for h in range(NH_C):
                            nc.tensor.matmul(
                                op_ps,
                                lhsT=ot_sb[h // 2][:, h % 2, 128 * s : 128 * (s + 1)],
                                rhs=wo_sb[:, h, 512 * db : 512 * (db + 1)],
                                start=h == 0, stop=h == NH_C - 1,
                            )
                        ob = outp.tile([128, 512], f16, name="ob")
                        if (s + db) % 2 == 0:
                            nc.vector.tensor_copy(out=ob, in_=op_ps)
                        else:
                            nc.scalar.copy(out=ob, in_=op_ps)
                        # alternate DMA queues so the final block's writes
                        # drain in parallel instead of serializing on one
                        # engine queue at kernel teardown
                        dma_eng = (
                            nc.gpsimd if (gp_outdma and (s + db) % 2 == 0)
                            else nc.sync
                        )
                        dma_eng.dma_start(
                            out=outd[
                                ts0 + 128 * s : ts0 + 128 * (s + 1),
                                512 * db : 512 * (db + 1),
                            ],
                            in_=ob,
                        )

            def rope(dst, raw, rot_ps, ts0):
                # dst = raw * cos + (R @ raw) * sin   (rot_ps already in PSUM)
                nc.vector.tensor_mul(dst, raw, cos_sb[:, ts0 : ts0 + 512])
                rtmp = work.tile([128, 512], f16, name="rtmp", bufs=2)
                nc.vector.tensor_mul(rtmp, rot_ps, sin_sb[:, ts0 : ts0 + 512])
                nc.vector.tensor_add(dst, dst, rtmp)

            pending_oproj = None
            for tb in range(tb_n):
                ts0 = tb * 512
                # ============ phase A: projections + RoPE for this t-block
                # q heads in two pair-passes, then k/v, so at most two 2-bank
                # PSUM tiles are alive at once.
                xts = []
                for chunk in range(nd // 2):
                    c2 = 2 * chunk
                    xt = xin.tile([128, 2, 512], f16, name="xt")
                    nc.sync.dma_start(
                        out=xt, in_=x_re[:, c2 : c2 + 2, ts0 : ts0 + 512]
                    )
                    xts.append(xt)
                    if tb == 0:
                        nc.sync.dma_start(
                            out=wq_sb[:, c2 : c2 + 2, :], in_=wq_re[:, c2 : c2 + 2, :]
                        )
                        if chunk == 0:
                            nc.sync.dma_start(out=bq_sb, in_=bqd[:, :])
                            nc.sync.dma_start(out=bk_sb, in_=bkd[:, :])
                            nc.sync.dma_start(out=bv_sb, in_=bvd[:, :])
                            nc.vector.memset(ones_f, 1.0)
                            nc.scalar.copy(out=ones_sb, in_=ones_f)
                            # one-time loads on the gpsimd queue so the sync
                            # queue stays dedicated to the xt/w stream
                            nc.gpsimd.dma_start(out=cos_sb, in_=cosd[:, :])
                            nc.gpsimd.dma_start(out=sin_sb, in_=sind[:, :])
                            nc.gpsimd.dma_start(out=rt_sb, in_=rtd[:, :])
                            nc.gpsimd.dma_start(out=mask_sb, in_=maskd[:, :])
                            nc.gpsimd.dma_start(out=id_sb, in_=identd[:, :])
                if tb == 0:
                    # wk/wv are first consumed ~15us in (after both q-pair
                    # passes); keep them off the startup critical path
                    for chunk in range(nd // 2):
                        c2 = 2 * chunk
                        nc.sync.dma_start(
                            out=wk_sb[:, c2 : c2 + 2, :], in_=wk_re[:, c2 : c2 + 2, :]
                        )
                        nc.sync.dma_start(
                            out=wv_sb[:, c2 : c2 + 2, :], in_=wv_re[:, c2 : c2 + 2, :]
                        )
                if tb == min(1, tb_n - 1):
                    # wo is first consumed by block 1's o_proj emission; defer
                    # its 2MB load out of the startup HBM contention window
                    wo_re = wod[:, :].rearrange("(h p) m -> p h m", p=128)
                    for h in range(NH_C):
                        nc.gpsimd.dma_start(
                            out=wo_sb[:, h : h + 1, :],
                            in_=wo_re[:, h : h + 1, :],
                        )

                # all projection matmuls first (q pair 0, q pair 1, then k/v
                # into a shared 2-bank tile) so the PE streams without waiting
                # on any ACT/DVE evacuation; rope chains run behind it.
                qt2 = {}
                for p in range(2):
                    qt2[p] = pb(f"qt2_{p}")
                    for chunk in range(nd // 2):
                        for j in range(2):
                            dt = 2 * chunk + j
                            first, last = dt == 0, dt == nd - 1
                            for hh in range(2):
                                h = 2 * p + hh
                                nc.tensor.matmul(
                                    qt2[p][:, hh, :],
                                    lhsT=wq_sb[:, dt, h * HS : (h + 1) * HS],
                                    rhs=xts[chunk][:, j, :],
                                    start=first, stop=last,
                                    skip_group_check=True,
                                )
                ktvt = pb("ktvt")
                for chunk in range(nd // 2):
                    for j in range(2):
                        dt = 2 * chunk + j
                        first, last = dt == 0, dt == nd - 1
                        nc.tensor.matmul(
                            ktvt[:, 0, :], lhsT=wk_sb[:, dt, :],
                            rhs=xts[chunk][:, j, :],
                            start=first, stop=last, skip_group_check=True,
                        )
                        nc.tensor.matmul(
                            ktvt[:, 1, :], lhsT=wv_sb[:, dt, :],
                            rhs=xts[chunk][:, j, :],
                            start=first, stop=last, skip_group_check=True,
                        )

                qf = qfp.tile([128, NH_C, 512], f16, name="qf")
                for p in range(2):
                    for hh in range(2):
                        h = 2 * p + hh
                        qraw = work.tile([128, 512], f16, name="qraw")
                        nc.scalar.activation(
                            out=qraw, in_=qt2[p][:, hh, :], func=ident_f,
                            bias=bq_sb[:, h : h + 1], scale=1.0,
                        )
                        rot_ps = b1("rot_ps")
                        nc.tensor.matmul(
                            rot_ps, lhsT=rt_sb, rhs=qraw, start=True, stop=True
                        )
                        rope(qf[:, h, :], qraw, rot_ps, ts0)

                kraw = work.tile([128, 512], f16, name="qraw")
                nc.scalar.activation(
                    out=kraw, in_=ktvt[:, 0, :], func=ident_f,
                    bias=bk_sb[:, 0:1], scale=1.0,
                )
                rot_ps = b1("rot_ps")
                nc.tensor.matmul(rot_ps, lhsT=rt_sb, rhs=kraw, start=True, stop=True)
                rope(kt_sb[:, ts0 : ts0 + 512], kraw, rot_ps, ts0)

                # v: bias (fp16 cast), then transpose to [t, hs] tiles
                vraw = work.tile([128, 512], f16, name="qraw")
                nc.scalar.activation(
                    out=vraw, in_=ktvt[:, 1, :], func=ident_f,
                    bias=bv_sb[:, 0:1], scale=1.0,
                )
                for s in range(4):
                    vt_tp = b1("vt_tp")
                    vt16 = vt_tp[:, 0:64].bitcast(f16)
                    nc.tensor.transpose(
                        vt16, vraw[:, 128 * s : 128 * (s + 1)], id_sb
                    )
                    nc.scalar.copy(out=v_sb[:, 4 * tb + s, :], in_=vt16)

                # previous block's o_proj: PE chews on it while ACT/DVE
                # finish this block's RoPE chain (qf not needed yet).
                if pending_oproj is not None:
                    emit_oproj(tb - 1, pending_oproj)

                # ============ phase B: attention for q-block jq == tb
                # Pair-merged: one S matmul / exp / PV matmul per head-pair
                # and k-tile.  Software-pipelined by one k-tile.
                ot_sb = {}
                imax = 4 * tb + 3
                for p in range(2):
                    ot2 = pb(f"ot2_{p}")
                    den = b1(f"den_{p}")
                    if dve_den:
                        acc = work.tile(
                            [128, 2, 512], f32, name="acc", bufs=2
                        )

                    def emit_pv_den(i, pt, c0, ot2=ot2, den=den):
                        first, last = i == 0, i == imax
                        for hh in range(2):
                            nc.tensor.matmul(
                                ot2[:, hh, c0:], lhsT=v_sb[:, i, :],
                                rhs=pt[:, hh, c0:],
                                start=first, stop=last, skip_group_check=True,
                            )
                        if not dve_den:
                            for hh in range(2):
                                nc.tensor.matmul(
                                    den[32 * hh : 32 * hh + 1, c0:],
                                    lhsT=ones_sb[:, 0:1],
                                    rhs=pt[:, hh, c0:],
                                    start=first, stop=last,
                                    skip_group_check=True,
                                )

                    prev = None
                    for i in range(imax + 1):
                        c0 = 128 * max(0, i - 4 * tb)
                        st2 = pb("st2")
                        for hh in range(2):
                            nc.tensor.matmul(
                                st2[:, hh, c0:],
                                lhsT=kt_sb[:, 128 * i : 128 * (i + 1)],
                                rhs=qf[:, 2 * p + hh, c0:],
                                start=True, stop=True, skip_group_check=True,
                            )
                        pt = ptp.tile([128, 2, 512], f16, name="pt")
                        nc.scalar.activation(
                            out=pt[:, :, c0:], in_=st2[:, :, c0:], func=exp_f,
                            scale=INV_SQRT_HS,
                        )
                        if i >= 4 * tb:
                            for hh in range(2):
                                nc.vector.tensor_mul(
                                    pt[:, hh, c0 : c0 + 128],
                                    pt[:, hh, c0 : c0 + 128],
                                    mask_sb,
                                )
                        if dve_den:
                            # running P column-sum on DVE: frees the PE from
                            # the 1-row denominator matmuls per k-tile
                            if i == 0:
                                nc.vector.tensor_copy(out=acc, in_=pt)
                            else:
                                nc.vector.tensor_add(
                                    acc[:, :, c0:], acc[:, :, c0:], pt[:, :, c0:]
                                )
                        if prev is not None:
                            emit_pv_den(*prev)
                        prev = (i, pt, c0)
                    emit_pv_den(*prev)
                    if dve_den:
                        # one pair of 1-row reductions over the accumulated
                        # P-sums instead of one per k-tile
                        acch = ptp.tile([128, 2, 512], f16, name="acc16")
                        nc.vector.tensor_copy(out=acch, in_=acc)
                        for hh in range(2):
                            nc.tensor.matmul(
                                den[32 * hh : 32 * hh + 1, :],
                                lhsT=ones_sb[:, 0:1],
                                rhs=acch[:, hh, :],
                                start=True, stop=True,
                                skip_group_check=True,
                            )

                    # evacuate O^T pair immediately (frees PSUM), then
                    # normalize in SBUF with slack: broadcast the raw
                    # denominator with a K=1 ones matmul, fast-approx
                    # reciprocal on DVE, multiply in place.
                    osb = otp.tile([128, 2, 512], f16, name="osb")
                    nc.scalar.copy(out=osb, in_=ot2)
                    denrow = work.tile([33, 512], f16, name="denrow", bufs=2)
                    nc.scalar.copy(out=denrow[0:1, :], in_=den[0:1, :])
                    nc.scalar.copy(out=denrow[32:33, :], in_=den[32:33, :])
                    for hh in range(2):
                        bc_ps = b1("bc_ps")
                        nc.tensor.matmul(
                            bc_ps,
                            lhsT=ones_sb[32 * hh : 32 * hh + 1, 0:128],
                            rhs=denrow[32 * hh : 32 * hh + 1, :],
                            start=True, stop=True,
                        )
                        bcr = work.tile([128, 512], f32, name="bcr", bufs=2)
                        nc.vector.reciprocal_approx_fast(out=bcr, in_=bc_ps)
                        nc.vector.tensor_mul(osb[:, hh, :], osb[:, hh, :], bcr)
                    ot_sb[p] = osb

                pending_oproj = ot_sb

            emit_oproj(tb_n - 1, pending_oproj)

    nc.compile()
    return nc


def shard_inputs(x, cos, sin, Wq, bq, Wkv, bkv, Wo, t=T):
    """Build the 8 per-core input maps (core c -> batch c//4, group c%4)."""
    f16 = np.float16
    f32 = np.float32
    hs = HS
    rot = np.zeros((hs, hs), f32)
    for i in range(hs // 2):
        rot[i, i + hs // 2] = -1.0
        rot[i + hs // 2, i] = 1.0
    r_t = np.ascontiguousarray(rot.T.astype(f16))
    mask_ut = np.triu(np.ones((128, 128), f16))
    ident = np.eye(128, dtype=f16)
    cos_t = np.ascontiguousarray(np.asarray(cos, f32).T.astype(f16))
    sin_t = np.ascontiguousarray(np.asarray(sin, f32).T.astype(f16))

    xts = [
        np.ascontiguousarray(np.asarray(x[b], f32).T.astype(f16))
        for b in range(x.shape[0])
    ]
    per_g = []
    for g in range(4):
        per_g.append(
            dict(
                wq_t=np.ascontiguousarray(
                    Wq[512 * g : 512 * g + 512].T.astype(f16)
                ),
                b_q=np.ascontiguousarray(
                    bq[512 * g : 512 * g + 512].reshape(4, 128).T.astype(f32)
                ),
                wk_t=np.ascontiguousarray(
                    Wkv[128 * g : 128 * g + 128].T.astype(f16)
                ),
                b_k=np.ascontiguousarray(
                    bkv[128 * g : 128 * g + 128].reshape(128, 1).astype(f32)
                ),
                wv_t=np.ascontiguousarray(
                    Wkv[512 + 128 * g : 512 + 128 * g + 128].T.astype(f16)
                ),
                b_v=np.ascontiguousarray(
                    bkv[512 + 128 * g : 512 + 128 * g + 128]
                    .reshape(128, 1)
                    .astype(f32)
                ),
                wo_t=np.ascontiguousarray(
                    Wo[:, 512 * g : 512 * g + 512].T.astype(f16)
                ),
            )
        )

    in_maps = []
    for c in range(4 * x.shape[0]):
        b, g = c // 4, c % 4
        m = dict(per_g[g])
        m.update(
            x_t=xts[b], cos_t=cos_t, sin_t=sin_t,
            r_t=r_t, mask_ut=mask_ut, ident=ident,
        )
        in_maps.append(m)
    return in_maps


def run_on_hw(in_maps, t=T, trace=False, **flags):
    from concourse.bass_utils import run_bass_kernel_spmd

    key = (t, tuple(sorted(flags.items())))
    if key not in _NC_CACHE:
        _NC_CACHE[key] = build_nc(t, **flags)
    nc = _NC_CACHE[key]
    res = run_bass_kernel_spmd(
        nc, in_maps, core_ids=list(range(len(in_maps))), trace=trace
    )
    return res


def kernel(x, cos, sin, Wq, bq, Wkv, bkv, Wo):
    x = np.asarray(x)
    in_maps = shard_inputs(
        x, np.asarray(cos), np.asarray(sin), np.asarray(Wq), np.asarray(bq),
        np.asarray(Wkv), np.asarray(bkv), np.asarray(Wo),
    )
    res = run_on_hw(in_maps, t=T, trace=False)
    out = np.zeros((B, T, D), np.float32)
    for c, rmap in enumerate(res.results):
        out[c // 4] += rmap["out"].astype(np.float32)
    return out
